# revision 1
# baseline (speedup 1.0000x reference)
"""AttentionPairBias kernel for Trainium2, 8-core SPMD.

Problem (reference.py):
  q = (s @ Wq + bq); k = k_in @ Wk; v = k_in @ Wv       (B,N,H,D)
  zn = LayerNorm(z) * ln_g + ln_b                        (B,N,N,CZ)
  bias = transpose(zn @ Wz) -> (B,H,N,N)
  g = sigmoid(s @ Wg)
  p = softmax_j(q.k/sqrt(D) + bias + maskterm)
  out = (g * (p @ v)) @ Wo

Sharding: 8 cores = (batch b in {0,1}) x (4 slices of 192 query rows i).
Each core computes out[b, i0:i0+192, :] completely (full key range
on-core), so the host only slices inputs / concatenates outputs.
Host-side prep is layout-only: slicing, transposes (sT, kinT, per-row zT).

Device math restructuring:
  LayerNorm folded into the Wz projection:
    bias[pair,h] = rstd[pair] * (z[pair,:] @ W''[:,h])  (+ per-(b,h,i) const
    dropped -- softmax over j is invariant to it; this absorbs ln_b and the
    mean term)
  W'[c,h] = ln_g[c]*Wz[c,h];  W''[c,h] = W'[c,h] - mean_c W'[:,h]
  rstd from per-pair sum(z) (ones-column riding the same matmul) and
  sum(z^2) (ones-matmul over squared z).
"""

import os
import numpy as np

B, N, CS, CZ, H, D = 2, 768, 1024, 128, 16, 64
NCORES = 8
IPC = N // 4            # 192 query rows per core
CT_host = CS // 128
LN_EPS = 1e-5
INV_SQRT_D = 0.125
INF = 1000000.0

_prog_cache = {}


def _build(ipc=IPC, mask_ones=True, repeat=1):
    import contextlib
    import concourse.bass as bass
    import concourse.tile as tile
    from concourse import bacc, mybir

    f32 = mybir.dt.float32
    bf16 = mybir.dt.bfloat16
    AF = mybir.ActivationFunctionType
    OP = mybir.AluOpType

    CT = CS // 128          # 8 c_s tiles
    JT = N // 128           # 6 j tiles
    assert ipc % 4 == 0
    itiles = []
    o = 0
    while o < ipc:
        isz = min(128, ipc - o)
        itiles.append((o, isz, 0))
        o += isz

    nc = bacc.Bacc("TRN2", target_bir_lowering=False, debug=False,
                   enable_asserts=False, num_devices=NCORES)

    # ---- DRAM I/O ----
    zT_d = nc.dram_tensor("zT", [ipc * CZ, N], f32, kind="ExternalInput")
    sT_d = nc.dram_tensor("sT", [CS, ipc], f32, kind="ExternalInput")
    kinT_d = nc.dram_tensor("kinT", [CS, N], f32, kind="ExternalInput")
    W_d = {w: nc.dram_tensor(w, [CS, CS], f32, kind="ExternalInput")
           for w in ("Wq", "Wk", "Wv", "Wg", "Wo")}
    Wz_d = nc.dram_tensor("Wz", [CZ, H], f32, kind="ExternalInput")
    lng_d = nc.dram_tensor("ln_g", [CZ, 1], f32, kind="ExternalInput")
    bqT_d = nc.dram_tensor("bqT", [128, CT], f32, kind="ExternalInput")
    if not mask_ones:
        mask_d = nc.dram_tensor("maskrow", [1, N], f32, kind="ExternalInput")
    outT_d = nc.dram_tensor("outT", [CS, ipc], f32, kind="ExternalOutput")

    with tile.TileContext(nc) as tc:
        # ---------------- persistent SBUF ----------------
        Wsb = {w: nc.alloc_sbuf_tensor(f"{w}_sb", [128, CT, CS], bf16)
               for w in ("Wq", "Wk", "Wv", "Wg", "Wo")}
        kinT_sb = nc.alloc_sbuf_tensor("kinT_sb", [128, CT, N], bf16)
        sT_sb = nc.alloc_sbuf_tensor("sT_sb", [128, CT, ipc], bf16)
        qT_sb = nc.alloc_sbuf_tensor("qT_sb", [128, CT, ipc], bf16)
        kT_sb = nc.alloc_sbuf_tensor("kT_sb", [128, CT, N], bf16)
        v_sb = nc.alloc_sbuf_tensor("v_sb", [128, JT, CS], bf16)
        gT_sb = nc.alloc_sbuf_tensor("gT_sb", [128, CT, ipc], bf16)
        goT_sb = nc.alloc_sbuf_tensor("goT_sb", [128, CT, 128], bf16)
        bqT_sb = nc.alloc_sbuf_tensor("bqT_sb", [128, CT], f32)
        Wz_sb = nc.alloc_sbuf_tensor("Wz_sb", [128, H], f32)
        lng_sb = nc.alloc_sbuf_tensor("lng_sb", [128, 1], f32)
        Wp_sb = nc.alloc_sbuf_tensor("Wp_sb", [128, H], f32)
        G_sb = nc.alloc_sbuf_tensor("G_sb", [128, H], f32)
        Grow_sb = nc.alloc_sbuf_tensor("Grow_sb", [1, H], f32)
        Waug = nc.alloc_sbuf_tensor("Waug", [128, 32], bf16)
        ones17 = nc.alloc_sbuf_tensor("ones17", [128, 32], bf16)
        id_sb = nc.alloc_sbuf_tensor("id_sb", [128, 128], bf16)
        eps_sb = nc.alloc_sbuf_tensor("eps_sb", [128, 1], f32)
        # per-i-tile working buffers
        P_nat = nc.alloc_sbuf_tensor("P_nat", [128, H, N], bf16)
        musq_nat = nc.alloc_sbuf_tensor("musq_nat", [128, 2, N], bf16)
        stat_a = nc.alloc_sbuf_tensor("stat_a", [128, N], f32)
        stat_b = nc.alloc_sbuf_tensor("stat_b", [128, N], f32)
        alpha = nc.alloc_sbuf_tensor("alpha", [128, N], bf16)
        den_sb = nc.alloc_sbuf_tensor("den_sb", [128, 2 * H], f32)
        rden_sb = nc.alloc_sbuf_tensor("rden_sb", [128, 2 * H], f32)
        if not mask_ones:
            mrow_sb = nc.alloc_sbuf_tensor("mrow_sb", [1, N], f32)
            mbias_sb = nc.alloc_sbuf_tensor("mbias_sb", [1, N], f32)
            mb_full = nc.alloc_sbuf_tensor("mb_full", [128, N], bf16)

        ctx = contextlib.ExitStack()
        with ctx:
            ps = ctx.enter_context(tc.tile_pool(name="ps", bufs=8, space="PSUM"))
            zpool = ctx.enter_context(tc.tile_pool(name="zs", bufs=3))
            tpool = ctx.enter_context(tc.tile_pool(name="ts", bufs=2))

            # ---------------- constant / weight prep ----------------
            nc.sync.dma_start(out=bqT_sb[:, :], in_=bqT_d.ap())
            nc.sync.dma_start(out=Wz_sb[:, :], in_=Wz_d.ap())
            nc.sync.dma_start(out=lng_sb[:, :], in_=lng_d.ap())
            nc.vector.memset(eps_sb[:, :], LN_EPS)
            nc.vector.memset(ones17[:, :], 1.0)
            nc.vector.memset(Waug[:, :], 0.0)
            from concourse.masks import make_identity
            make_identity(nc, id_sb[:, :])
            # W' = ln_g*Wz ; G = colsum(W') ; W'' = W' - G/128
            nc.vector.tensor_scalar_mul(out=Wp_sb[:, :], in0=Wz_sb[:, :],
                                        scalar1=lng_sb[:, :])
            nc.vector.tensor_copy(out=Waug[:, 0:H], in_=Wp_sb[:, :])
            nc.vector.memset(Waug[:, H:H + 1], 1.0)
            G_ps = ps.tile([1, H], f32, tag="bank")
            nc.tensor.matmul(out=G_ps[:, :], lhsT=Waug[:, H:H + 1],
                             rhs=Waug[:, 0:H], start=True, stop=True)
            nc.vector.tensor_copy(out=Grow_sb[:, :], in_=G_ps[:, :])
            Grow_dram = nc.dram_tensor("Grow_dram", [H], f32, kind="Internal")
            nc.sync.dma_start(out=Grow_dram.ap(), in_=Grow_sb[:, :])
            nc.sync.dma_start(
                out=G_sb[:, :],
                in_=bass.AP(tensor=Grow_dram, offset=0, ap=[[0, 128], [1, H]]))
            nc.vector.scalar_tensor_tensor(out=Wp_sb[:, :], in0=G_sb[:, :],
                                           scalar=-1.0 / 128.0, in1=Wp_sb[:, :],
                                           op0=OP.mult, op1=OP.add)
            nc.vector.tensor_copy(out=Waug[:, 0:H], in_=Wp_sb[:, :])
            if not mask_ones:
                nc.sync.dma_start(out=mrow_sb[:, :], in_=mask_d.ap())
                nc.vector.tensor_scalar(out=mbias_sb[:, :], in0=mrow_sb[:, :],
                                        scalar1=1.0, scalar2=INF,
                                        op0=OP.subtract, op1=OP.mult)
                mb_dram = nc.dram_tensor("mb_dram", [N], f32, kind="Internal")
                nc.sync.dma_start(out=mb_dram.ap(), in_=mbias_sb[:, :])
                nc.gpsimd.dma_start(
                    out=mb_full[:, :],
                    in_=bass.AP(tensor=mb_dram, offset=0, ap=[[0, 128], [1, N]]))

            # weight / activation loads (SWDGE casts fp32->bf16)
            for w in ("Wq", "Wk", "Wv", "Wg", "Wo"):
                nc.gpsimd.dma_start(
                    out=Wsb[w][:, :, :],
                    in_=W_d[w].ap().rearrange("(t p) f -> p t f", p=128))
            nc.gpsimd.dma_start(
                out=kinT_sb[:, :, :],
                in_=kinT_d.ap().rearrange("(t p) j -> p t j", p=128))
            nc.gpsimd.dma_start(
                out=sT_sb[:, :, :],
                in_=sT_d.ap().rearrange("(t p) i -> p t i", p=128))

            for _rep in range(repeat):
                # ---------------- projections ----------------
                for f in range(CT):
                    g_ps = ps.tile([128, ipc], f32, tag="bank")
                    for c in range(CT):
                        nc.tensor.matmul(out=g_ps[:, :],
                                         lhsT=Wsb["Wg"][:, c, 128 * f:128 * (f + 1)],
                                         rhs=sT_sb[:, c, :],
                                         start=(c == 0), stop=(c == CT - 1))
                    nc.scalar.activation(out=gT_sb[:, f, :], in_=g_ps[:, :], func=AF.Sigmoid)
                for f in range(CT):
                    q_ps = ps.tile([128, ipc], f32, tag="bank")
                    for c in range(CT):
                        nc.tensor.matmul(out=q_ps[:, :],
                                         lhsT=Wsb["Wq"][:, c, 128 * f:128 * (f + 1)],
                                         rhs=sT_sb[:, c, :],
                                         start=(c == 0), stop=(c == CT - 1))
                    nc.vector.tensor_scalar_add(out=qT_sb[:, f, :], in0=q_ps[:, :],
                                                scalar1=bqT_sb[:, f:f + 1])
                for f in range(CT):
                    for hf in range(2):
                        k_ps = ps.tile([128, 384], f32, tag="bank")
                        for c in range(CT):
                            nc.tensor.matmul(out=k_ps[:, :],
                                             lhsT=Wsb["Wk"][:, c, 128 * f:128 * (f + 1)],
                                             rhs=kinT_sb[:, c, 384 * hf:384 * (hf + 1)],
                                             start=(c == 0), stop=(c == CT - 1))
                        nc.vector.tensor_copy(out=kT_sb[:, f, 384 * hf:384 * (hf + 1)],
                                              in_=k_ps[:, :])
                for jt in range(JT):
                    for hf in range(2):
                        v_ps = ps.tile([128, 512], f32, tag="bank")
                        for c in range(CT):
                            nc.tensor.matmul(out=v_ps[:, :],
                                             lhsT=kinT_sb[:, c, 128 * jt:128 * (jt + 1)],
                                             rhs=Wsb["Wv"][:, c, 512 * hf:512 * (hf + 1)],
                                             start=(c == 0), stop=(c == CT - 1))
                        nc.vector.tensor_copy(out=v_sb[:, jt, 512 * hf:512 * (hf + 1)],
                                              in_=v_ps[:, :])

                # ---------------- main loop over i-tiles ----------------
                for (i0, isz, poff) in itiles:
                    # ---- z stream: 4 i-rows per DMA (host-pretransposed zT) ----
                    for r4 in range(i0, i0 + isz, 4):
                        zTt = zpool.tile([128, 4, N], bf16, tag="zTt")
                        zsrc = bass.AP(
                            tensor=zT_d,
                            offset=r4 * CZ * N,
                            ap=[[N, 128], [CZ * N, 4], [1, N]],
                        )
                        nc.gpsimd.dma_start(out=zTt[:, :, :], in_=zsrc)
                        Ppk = [ps.tile([128, 384], f32, tag="bank", name=f"Ppk{_h}")
                               for _h in range(2)]
                        Spk = [ps.tile([128, 384], f32, tag="bank", name=f"Spk{_h}")
                               for _h in range(2)]
                        for rl in range(4):
                            zsqT = zpool.tile([128, N], bf16, tag="zsqT")
                            if rl % 2 == 0:
                                nc.scalar.activation(out=zsqT[:, :], in_=zTt[:, rl, :],
                                                     func=AF.Square)
                            else:
                                nc.vector.scalar_tensor_tensor(
                                    out=zsqT[:, :], in0=zTt[:, rl, :], scalar=1.0,
                                    in1=zTt[:, rl, :], op0=OP.mult, op1=OP.mult)
                            for hf in range(2):
                                nc.tensor.matmul(out=Ppk[hf][32 * rl:32 * rl + 32, :],
                                                 lhsT=Waug[:, :],
                                                 rhs=zTt[:, rl, 384 * hf:384 * (hf + 1)],
                                                 start=True, stop=True,
                                                 tile_position=(0, 32 * rl))
                                nc.tensor.matmul(out=Spk[hf][32 * rl:32 * rl + 32, :],
                                                 lhsT=ones17[:, :],
                                                 rhs=zsqT[:, 384 * hf:384 * (hf + 1)],
                                                 start=True, stop=True,
                                                 tile_position=(0, 32 * rl))
                        PSsb = tpool.tile([128, 2, N], bf16, tag="PSsb")
                        for hf in range(2):
                            nc.vector.tensor_copy(out=PSsb[:, 0, 384 * hf:384 * (hf + 1)],
                                                  in_=Ppk[hf][:, :])
                            nc.scalar.copy(out=PSsb[:, 1, 384 * hf:384 * (hf + 1)],
                                           in_=Spk[hf][:, :])
                        for rl in range(4):
                            li = r4 + rl - i0 + poff
                            eng_p = nc.sync if rl % 2 == 0 else nc.scalar
                            eng_m = nc.scalar if rl % 2 == 0 else nc.sync
                            eng_p.dma_start(out=P_nat[li:li + 1, :, :],
                                            in_=PSsb[32 * rl:32 * rl + H, 0, :])
                            eng_m.dma_start(out=musq_nat[li:li + 1, :, :],
                                            in_=PSsb[32 * rl + H:32 * rl + H + 1, :, :])

                    # ---- stats -> alpha ; bias = alpha * P (in place) ----
                    sl = slice(poff, poff + isz)
                    nc.vector.scalar_tensor_tensor(
                        out=stat_a[sl, :], in0=musq_nat[sl, 0, :],
                        scalar=1.0 / (128.0 * 128.0), in1=musq_nat[sl, 0, :],
                        op0=OP.mult, op1=OP.mult)
                    nc.vector.scalar_tensor_tensor(
                        out=stat_b[sl, :], in0=musq_nat[sl, 1, :], scalar=1.0 / 128.0,
                        in1=stat_a[sl, :], op0=OP.mult, op1=OP.subtract)
                    # rstd = exp(-0.5*ln(var+eps)) -- keeps ACT on one table set
                    nc.scalar.activation(out=stat_b[sl, :], in_=stat_b[sl, :],
                                         func=AF.Ln, bias=eps_sb[sl, :], scale=1.0)
                    nc.scalar.activation(out=alpha[sl, :], in_=stat_b[sl, :],
                                         func=AF.Exp, scale=-0.5)

                    # ---- attention ----
                    for h in range(H):
                        hp, off = h // 2, 64 * (h % 2)
                        nc.vector.scalar_tensor_tensor(
                            out=P_nat[sl, h, :], in0=P_nat[sl, h, :], scalar=1.0,
                            in1=alpha[sl, :], op0=OP.mult, op1=OP.mult)
                        if not mask_ones:
                            nc.vector.tensor_add(out=P_nat[sl, h, :],
                                                 in0=P_nat[sl, h, :],
                                                 in1=mb_full[sl, :])
                        qk0 = ps.tile([128, 384], f32, tag="bank")
                        qk1 = ps.tile([128, 384], f32, tag="bank")
                        p_sb = tpool.tile([128, N], bf16, tag="p_sb")
                        for hf, qk in ((0, qk0), (1, qk1)):
                            nc.tensor.matmul(out=qk[sl, :],
                                             lhsT=qT_sb[off:off + 64, hp, i0:i0 + isz],
                                             rhs=kT_sb[off:off + 64, hp, 384 * hf:384 * (hf + 1)],
                                             start=True, stop=True,
                                             tile_position=(0, poff) if poff else None)
                            nc.vector.scalar_tensor_tensor(
                                out=qk[sl, :], in0=qk[sl, :], scalar=INV_SQRT_D,
                                in1=P_nat[sl, h, 384 * hf:384 * (hf + 1)],
                                op0=OP.mult, op1=OP.add)
                            nc.scalar.activation(out=p_sb[sl, 384 * hf:384 * (hf + 1)],
                                                 in_=qk[sl, :], func=AF.Exp,
                                                 accum_out=den_sb[sl, 2 * h + hf:2 * h + hf + 1])
                        nc.vector.tensor_add(out=den_sb[sl, 2 * h:2 * h + 1],
                                             in0=den_sb[sl, 2 * h:2 * h + 1],
                                             in1=den_sb[sl, 2 * h + 1:2 * h + 2])
                        nc.vector.reciprocal(out=rden_sb[sl, h:h + 1],
                                             in_=den_sb[sl, 2 * h:2 * h + 1])
                        nc.vector.tensor_scalar_mul(out=p_sb[sl, :], in0=p_sb[sl, :],
                                                    scalar1=rden_sb[sl, h:h + 1])
                        pT_ps = ps.tile([128, JT, 128], bf16, tag="bank")
                        for jt in range(JT):
                            nc.tensor.transpose(out=pT_ps[:, jt, :isz],
                                                in_=p_sb[sl, 128 * jt:128 * (jt + 1)],
                                                identity=id_sb[sl, sl])
                        pT = tpool.tile([128, JT, 128], bf16, tag="pT")
                        nc.vector.tensor_copy(out=pT[:, :, :isz], in_=pT_ps[:, :, :isz])
                        if h % 2 == 0:
                            oT_ps = ps.tile([128, 128], f32, tag="bank")
                        for jt in range(JT):
                            nc.tensor.matmul(out=oT_ps[off:off + 64, :isz],
                                             lhsT=v_sb[:, jt, 64 * h:64 * (h + 1)],
                                             rhs=pT[:, jt, :isz],
                                             start=(jt == 0), stop=(jt == JT - 1))
                        if h % 2 == 1:
                            nc.vector.tensor_mul(out=goT_sb[:, hp, :isz],
                                                 in0=oT_ps[:, :isz],
                                                 in1=gT_sb[:, hp, i0:i0 + isz])

                    # ---- output projection ----
                    for f in range(CT):
                        o_ps = ps.tile([128, 128], f32, tag="bank")
                        for c in range(CT):
                            nc.tensor.matmul(out=o_ps[:, :isz],
                                             lhsT=Wsb["Wo"][:, c, 128 * f:128 * (f + 1)],
                                             rhs=goT_sb[:, c, :isz],
                                             start=(c == 0), stop=(c == CT - 1))
                        ot = tpool.tile([128, 128], f32, tag="ot")
                        nc.vector.tensor_copy(out=ot[:, :isz], in_=o_ps[:, :isz])
                        odst = bass.AP(tensor=outT_d, offset=128 * f * ipc + i0,
                                       ap=[[ipc, 128], [1, isz]])
                        nc.sync.dma_start(out=odst, in_=ot[:, :isz])
    nc.compile()
    return nc


def _get_prog(ipc=IPC, mask_ones=True):
    key = (ipc, mask_ones)
    if key not in _prog_cache:
        _prog_cache[key] = _build(ipc, mask_ones)
    return _prog_cache[key]


def _in_maps(s, z, mask, k_in, Wq, bq, Wk, Wv, Wg, ln_g, ln_b, Wz, Wo, ipc=IPC):
    del ln_b  # constant along j after softmax -> drops out exactly
    maps = []
    nsl = NCORES // B
    for c in range(NCORES):
        b, t = divmod(c, nsl)
        i0 = t * ipc
        m = {
            "zT": np.ascontiguousarray(
                z[b, i0:i0 + ipc].transpose(0, 2, 1)).reshape(ipc * CZ, N),
            "sT": np.ascontiguousarray(s[b, i0:i0 + ipc].T),
            "kinT": np.ascontiguousarray(k_in[b].T),
            "Wq": Wq, "Wk": Wk, "Wv": Wv, "Wg": Wg, "Wo": Wo,
            "Wz": np.ascontiguousarray(Wz),
            "ln_g": np.ascontiguousarray(ln_g.reshape(CZ, 1)),
            "bqT": np.ascontiguousarray(bq.reshape(CT_host, 128).T),
        }
        if not bool(np.all(mask == 1.0)):
            m["maskrow"] = np.ascontiguousarray(mask[b].reshape(1, N))
        maps.append({k: np.asarray(v, dtype=np.float32) for k, v in m.items()})
    return maps


def kernel(**inputs):
    from concourse.bass_utils import run_bass_kernel_spmd
    mask_ones = bool(np.all(np.asarray(inputs["mask"]) == 1.0))
    nc = _get_prog(IPC, mask_ones)
    maps = _in_maps(**{k: np.asarray(v) for k, v in inputs.items()})
    trace = os.environ.get("KBENCH_TRACE", "") == "1"
    res = run_bass_kernel_spmd(nc, maps, core_ids=list(range(NCORES)), trace=trace)
    out = np.empty((B, N, CS), dtype=np.float32)
    nsl = NCORES // B
    for c in range(NCORES):
        b, t = divmod(c, nsl)
        out[b, t * IPC:(t + 1) * IPC, :] = res.results[c]["outT"].T
    if trace:
        print("HW exec time:", res.exec_time_ns, "ns")
    return out



# revision 21
# speedup vs baseline: 1.3874x; 1.3874x over previous
"""AttentionPairBias kernel for Trainium2, 8-core SPMD.

  q = (s @ Wq + bq); k = k_in @ Wk; v = k_in @ Wv          (B,N,H,D)
  bias = transpose(LayerNorm(z) @ Wz) -> (B,H,N,N)
  g = sigmoid(s @ Wg)
  p = softmax_j(q.k/sqrt(D) + bias + maskterm)
  out = (g * (p @ v)) @ Wo

Sharding: 8 cores = (batch b in {0,1}) x (4 slices of 192 query rows i).
Each core computes out[b, i0:i0+192, :] completely.  Inputs are uploaded
pre-cast to bf16 (halves HBM traffic; adds ~0.4% rel err vs 2e-2 budget).

Device structure per core:
  - LayerNorm folded into the Wz projection (W'' = ln_g*Wz - colmean/128;
    ln_b and per-row constants drop out of softmax_j).  Per 4 z-rows, one
    pair of PSUM tiles holds 32-partition strips, strip rl carrying:
      rows 0..15 = W''^T z (pre-bias P), row 16 = sum_c z, row 17 = sum_c z^2
    (P/sumz via lhsT=Waug[:,0:17] over z; sum z^2 via lhsT=ones over z^2).
  - PSUM strip -> SBUF bf16 -> XBAR dma-transpose into P_T[j | g, strip],
    so softmax runs with j on partitions:
      * p[j,i] is directly the PV lhsT operand: no PE transposes of p
      * logits = qk/8 + alpha*P via strided DVE/Pool ops
      * softmax denominator rides a ones-column inside vaug (even heads
        [v|1] -> den at psum row 64; odd heads [1|v] based at row 63)
  - normalization by the denominator happens post-PV (196K elems instead
    of 2.4M), fused with the sigmoid gate via a DRAM-broadcast of 1/den.
  - attention runs in two 96-row blocks so block 0 overlaps the z stream
    of block 1; the Wo projection of block 0 overlaps block 1 attention.
"""

import os
import sys
import types

import numpy as np
import ml_dtypes

B, N, CS, CZ, H, D = 2, 768, 1024, 128, 16, 64
NCORES = 8
IPC = N // 4            # 192 query rows per core
NBLK = 2
BLK = IPC // NBLK       # 96 rows per attention block
GRP = 4                 # z rows per PSUM-strip group
GPB = BLK // GRP        # 24 groups per block
NG = IPC // GRP         # 48 groups
JT = N // 128           # 6 j tiles
CT = CS // 128          # 8 c_s tiles
E = D + 1               # 65: head slot width in vaug (v + ones column)
LN_EPS = 1e-5
INV_SQRT_D = 0.125
INF = 1000000.0

_prog_cache = {}
BF16 = ml_dtypes.bfloat16


def _install_trace_shim():
    """Provide antenv.axon_hooks (NTFF profiling) if the image lacks it."""
    try:
        import antenv.axon_hooks  # noqa: F401
        return
    except ImportError:
        pass
    try:
        import trn_agent_boot.trn_boot as tb
        hook = tb._ntff_profile_via_ctypes('/opt/axon/libaxon_pjrt.so')
        mod = types.ModuleType("antenv.axon_hooks")
        mod.get_axon_ntff_profile_hook = lambda: hook
        sys.modules["antenv.axon_hooks"] = mod
    except Exception:
        pass


def _build(mask_ones=True):
    import contextlib
    import concourse.bass as bass
    import concourse.tile as tile
    from concourse import bacc, mybir

    f32 = mybir.dt.float32
    bf16 = mybir.dt.bfloat16
    AF = mybir.ActivationFunctionType
    OP = mybir.AluOpType

    nc = bacc.Bacc("TRN2", target_bir_lowering=False, debug=False,
                   enable_asserts=False, num_devices=NCORES)

    # ---- DRAM I/O (bf16 inputs pre-cast on host) ----
    zT_d = nc.dram_tensor("zT", [IPC * CZ, N], bf16, kind="ExternalInput")
    sT_d = nc.dram_tensor("sT", [CS, IPC], bf16, kind="ExternalInput")
    kinT_d = nc.dram_tensor("kinT", [CS, N], bf16, kind="ExternalInput")
    W_d = {w: nc.dram_tensor(w, [CS, CS], bf16, kind="ExternalInput")
           for w in ("Wq", "Wk", "Wv", "Wg", "Wo")}
    Wz_d = nc.dram_tensor("Wz", [CZ, H], f32, kind="ExternalInput")
    lng_d = nc.dram_tensor("ln_g", [CZ, 1], f32, kind="ExternalInput")
    bqT_d = nc.dram_tensor("bqT", [128, CT], f32, kind="ExternalInput")
    if not mask_ones:
        mask_d = nc.dram_tensor("maskrow", [1, N], f32, kind="ExternalInput")
    outT_d = nc.dram_tensor("outT", [CS, IPC], f32, kind="ExternalOutput")

    with tile.TileContext(nc) as tc:
        # ---------------- persistent SBUF ----------------
        Waug = nc.alloc_sbuf_tensor("Waug", [128, 32], bf16)
        # Saug: zeros except col 17 = ones; the z^2 matmul writes strip rows
        # 0..17 (rows 0..16 zero, row 17 = sum z^2) BEFORE the P matmul
        # overwrites rows 0..16 with P / sum z.
        Saug = nc.alloc_sbuf_tensor("Saug", [128, 18], bf16)
        Wz_sb = nc.alloc_sbuf_tensor("Wz_sb", [128, H], f32)
        lng_sb = nc.alloc_sbuf_tensor("lng_sb", [128, 1], f32)
        Wp_sb = nc.alloc_sbuf_tensor("Wp_sb", [128, H], f32)
        G_sb = nc.alloc_sbuf_tensor("G_sb", [128, H], f32)
        Grow_sb = nc.alloc_sbuf_tensor("Grow_sb", [1, H], f32)
        bqT_sb = nc.alloc_sbuf_tensor("bqT_sb", [128, CT], f32)
        eps_sb = nc.alloc_sbuf_tensor("eps_sb", [128, 1], f32)

        sT_sb = nc.alloc_sbuf_tensor("sT_sb", [128, CT, IPC], bf16)
        kinT_sb = nc.alloc_sbuf_tensor("kinT_sb", [128, CT, N], bf16)
        qT_sb = nc.alloc_sbuf_tensor("qT_sb", [128, CT, IPC], bf16)
        gT_sb = nc.alloc_sbuf_tensor("gT_sb", [128, CT, IPC], bf16)
        kT_sb = nc.alloc_sbuf_tensor("kT_sb", [128, CT, N], bf16)
        vaug = nc.alloc_sbuf_tensor("vaug", [128, JT, H * E], bf16)
        # P_T[j | jt, i, strip]: strip col h in 0..15 = head-h pre-bias P,
        # col 16 = sum_c z, col 17 = sum_c z^2 (stats consumed in j-layout)
        P_T = nc.alloc_sbuf_tensor("P_T", [128, JT, IPC, 32], bf16)
        alpha_T = nc.alloc_sbuf_tensor("alpha_T", [128, JT, IPC], bf16)
        stat_sb = nc.alloc_sbuf_tensor("stat_sb", [128, JT, BLK], f32)
        id_sb = nc.alloc_sbuf_tensor("id_sb", [128, 128], bf16)
        den_sb = nc.alloc_sbuf_tensor("den_sb", [128, H, BLK], f32)
        o_sb = nc.alloc_sbuf_tensor("o_sb", [64, H, BLK], bf16)
        rden_t = nc.alloc_sbuf_tensor("rden_t", [128, CT, BLK], f32)
        rden_dram = nc.dram_tensor("rden_dram", [NBLK, H * BLK], f32,
                                   kind="Internal")
        Grow_dram = nc.dram_tensor("Grow_dram", [H], f32, kind="Internal")
        if not mask_ones:
            mrow_sb = nc.alloc_sbuf_tensor("mrow_sb", [1, N], f32)
            mb_row = nc.alloc_sbuf_tensor("mb_row", [1, N], f32)
            mb_dram = nc.dram_tensor("mb_dram", [N], f32, kind="Internal")
            mb_T = nc.alloc_sbuf_tensor("mb_T", [128, JT], f32)

        ctx = contextlib.ExitStack()
        with ctx:
            ps = ctx.enter_context(tc.tile_pool(name="ps", bufs=8,
                                                space="PSUM"))
            zpool = ctx.enter_context(tc.tile_pool(name="zs", bufs=3))
            sqpool = ctx.enter_context(tc.tile_pool(name="sq", bufs=2))
            cpool = ctx.enter_context(tc.tile_pool(name="cs", bufs=4))
            apool = ctx.enter_context(tc.tile_pool(name="apl", bufs=3))
            wpool = ctx.enter_context(tc.tile_pool(name="wp", bufs=1))
            opool = ctx.enter_context(tc.tile_pool(name="op", bufs=3))

            # ---------------- constants / Waug prep ----------------
            nc.sync.dma_start(out=bqT_sb[:, :], in_=bqT_d.ap())
            nc.sync.dma_start(out=Wz_sb[:, :], in_=Wz_d.ap())
            nc.sync.dma_start(out=lng_sb[:, :], in_=lng_d.ap())
            nc.vector.memset(eps_sb[:, :], LN_EPS)
            nc.vector.memset(Saug[:, :], 0.0)
            nc.vector.memset(Saug[:, 17:18], 1.0)
            nc.vector.memset(Waug[:, :], 0.0)
            nc.vector.memset(vaug[:, :, :], 1.0)
            from concourse.masks import make_identity
            make_identity(nc, id_sb[:, :])
            # W' = ln_g*Wz ; G = colsum(W') ; W'' = W' - G/128
            nc.vector.tensor_scalar_mul(out=Wp_sb[:, :], in0=Wz_sb[:, :],
                                        scalar1=lng_sb[:, :])
            nc.vector.tensor_copy(out=Waug[:, 0:H], in_=Wp_sb[:, :])
            nc.vector.memset(Waug[:, H:H + 1], 1.0)
            G_ps = ps.tile([1, H], f32, tag="bank")
            nc.tensor.matmul(out=G_ps[:, :], lhsT=Waug[:, H:H + 1],
                             rhs=Waug[:, 0:H], start=True, stop=True)
            nc.vector.tensor_copy(out=Grow_sb[:, :], in_=G_ps[:, :])
            nc.sync.dma_start(out=Grow_dram.ap(), in_=Grow_sb[:, :])
            nc.sync.dma_start(
                out=G_sb[:, :],
                in_=bass.AP(tensor=Grow_dram, offset=0, ap=[[0, 128], [1, H]]))
            nc.vector.scalar_tensor_tensor(out=Wp_sb[:, :], in0=G_sb[:, :],
                                           scalar=-1.0 / 128.0,
                                           in1=Wp_sb[:, :],
                                           op0=OP.mult, op1=OP.add)
            nc.vector.tensor_copy(out=Waug[:, 0:H], in_=Wp_sb[:, :])
            # Waug col 16 stays all-ones: sum_c z rides the P matmul.
            if not mask_ones:
                nc.sync.dma_start(out=mrow_sb[:, :], in_=mask_d.ap())
                nc.vector.tensor_scalar(out=mb_row[:, :], in0=mrow_sb[:, :],
                                        scalar1=1.0, scalar2=INF,
                                        op0=OP.subtract, op1=OP.mult)
                nc.sync.dma_start(out=mb_dram.ap(), in_=mb_row[:, :])
                nc.sync.dma_start(
                    out=mb_T[:, :],
                    in_=bass.AP(tensor=mb_dram, offset=0,
                                ap=[[1, 128], [128, JT]]))

            # ---------------- activations / projections ----------------
            nc.scalar.dma_start(
                out=sT_sb[:, :, :],
                in_=sT_d.ap().rearrange("(t p) i -> p t i", p=128))
            nc.scalar.dma_start(
                out=kinT_sb[:, :, :],
                in_=kinT_d.ap().rearrange("(t p) j -> p t j", p=128))

            def load_w(w):
                t = wpool.tile([128, CT, CS], bf16, tag="w")
                nc.sync.dma_start(
                    out=t[:, :, :],
                    in_=W_d[w].ap().rearrange("(t p) f -> p t f", p=128))
                return t

            Wg_sb = load_w("Wg")
            for f in range(CT):
                g_ps = ps.tile([128, IPC], f32, tag="bank")
                for c in range(CT):
                    nc.tensor.matmul(out=g_ps[:, :],
                                     lhsT=Wg_sb[:, c, 128 * f:128 * (f + 1)],
                                     rhs=sT_sb[:, c, :],
                                     start=(c == 0), stop=(c == CT - 1))
                nc.scalar.activation(out=gT_sb[:, f, :], in_=g_ps[:, :],
                                     func=AF.Sigmoid)
            Wq_sb = load_w("Wq")
            for f in range(CT):
                q_ps = ps.tile([128, IPC], f32, tag="bank")
                for c in range(CT):
                    nc.tensor.matmul(out=q_ps[:, :],
                                     lhsT=Wq_sb[:, c, 128 * f:128 * (f + 1)],
                                     rhs=sT_sb[:, c, :],
                                     start=(c == 0), stop=(c == CT - 1))
                nc.vector.tensor_scalar_add(out=qT_sb[:, f, :], in0=q_ps[:, :],
                                            scalar1=bqT_sb[:, f:f + 1])
            Wk_sb = load_w("Wk")
            for f in range(CT):
                for hf in range(2):
                    k_ps = ps.tile([128, 384], f32, tag="bank")
                    for c in range(CT):
                        nc.tensor.matmul(
                            out=k_ps[:, :],
                            lhsT=Wk_sb[:, c, 128 * f:128 * (f + 1)],
                            rhs=kinT_sb[:, c, 384 * hf:384 * (hf + 1)],
                            start=(c == 0), stop=(c == CT - 1))
                    if (f + hf) % 2 == 0:
                        nc.vector.tensor_copy(
                            out=kT_sb[:, f, 384 * hf:384 * (hf + 1)],
                            in_=k_ps[:, :])
                    else:
                        nc.scalar.copy(
                            out=kT_sb[:, f, 384 * hf:384 * (hf + 1)],
                            in_=k_ps[:, :])
            Wv_sb = load_w("Wv")
            for jt in range(JT):
                for vh in range(2):
                    v_ps = ps.tile([128, 512], f32, tag="bank")
                    for c in range(CT):
                        nc.tensor.matmul(
                            out=v_ps[:, :],
                            lhsT=kinT_sb[:, c, 128 * jt:128 * (jt + 1)],
                            rhs=Wv_sb[:, c, 512 * vh:512 * (vh + 1)],
                            start=(c == 0), stop=(c == CT - 1))
                    # scatter 8 heads into vaug slots [v | ones]
                    base = 8 * vh * E
                    dst = vaug[:, jt, base:base + 8 * E].rearrange(
                        "p (e c) -> p e c", c=E)
                    src = v_ps[:, :].rearrange("p (e c) -> p e c", c=D)
                    nc.vector.tensor_copy(out=dst[:, :, 0:D], in_=src[:, :, :])

            Wo_sb = load_w("Wo")   # prefetched during z stream

            # ---------------- z stream ----------------
            def z_group(g):
                blk = g // GPB
                zTt = zpool.tile([128, GRP, N], bf16, tag="zTt")
                zsrc = bass.AP(tensor=zT_d, offset=(GRP * g) * CZ * N,
                               ap=[[N, 128], [CZ * N, GRP], [1, N]])
                (nc.sync if g % 2 == 0 else nc.scalar).dma_start(
                    out=zTt[:, :, :], in_=zsrc)
                zsq = sqpool.tile([128, GRP, N], bf16, tag="zsq")
                if g % 2 == 0:
                    nc.vector.tensor_mul(out=zsq[:, :, :], in0=zTt[:, :, :],
                                         in1=zTt[:, :, :])
                else:
                    nc.scalar.activation(out=zsq[:, :, :], in_=zTt[:, :, :],
                                         func=AF.Square)
                Ppk = [ps.tile([128, 384], f32, tag="bank", name=f"Ppk{hf}")
                       for hf in range(2)]
                # z^2 matmuls first (rows 0..17, row 17 = sum z^2), then P
                # overwrites rows 0..16 (row 16 = sum z); grouping by lhsT
                # keeps the stationary weights loaded.
                for hf in range(2):
                    for rl in range(GRP):
                        nc.tensor.matmul(
                            out=Ppk[hf][32 * rl:32 * rl + H + 2, :],
                            lhsT=Saug[:, :],
                            rhs=zsq[:, rl, 384 * hf:384 * (hf + 1)],
                            start=True, stop=True,
                            tile_position=(0, 32 * rl))
                for hf in range(2):
                    for rl in range(GRP):
                        nc.tensor.matmul(
                            out=Ppk[hf][32 * rl:32 * rl + H + 1, :],
                            lhsT=Waug[:, 0:H + 1],
                            rhs=zTt[:, rl, 384 * hf:384 * (hf + 1)],
                            start=True, stop=True,
                            tile_position=(0, 32 * rl))
                PT_ps = ps.tile([128, JT, 128], bf16, tag="bank", name="PT_ps")
                for hf in range(2):
                    PSsb = cpool.tile([128, 384], bf16, tag="PSsb")
                    nc.scalar.copy(out=PSsb[:, :], in_=Ppk[hf][:, :])
                    # PE transpose -> PT_ps[j | jt, strip]
                    for jtl in range(3):
                        nc.tensor.transpose(
                            out=PT_ps[:, 3 * hf + jtl, :],
                            in_=PSsb[:, 128 * jtl:128 * (jtl + 1)],
                            identity=id_sb[:, :])
                nc.vector.tensor_copy(
                    out=P_T[:, :, GRP * g:GRP * (g + 1), :],
                    in_=PT_ps[:, :, :])

            # ---------------- attention block ----------------
            def attention(blk):
                i0 = BLK * blk
                isl = slice(i0, i0 + BLK)
                # alpha = rstd per (i, j) computed in j-layout from P_T
                # cols 16 (sum z) and 17 (sum z^2)
                nc.vector.scalar_tensor_tensor(
                    out=stat_sb[:, :, :], in0=P_T[:, :, isl, 16],
                    scalar=1.0 / (128.0 * 128.0), in1=P_T[:, :, isl, 16],
                    op0=OP.mult, op1=OP.mult)
                nc.vector.scalar_tensor_tensor(
                    out=stat_sb[:, :, :], in0=P_T[:, :, isl, 17],
                    scalar=1.0 / 128.0, in1=stat_sb[:, :, :],
                    op0=OP.mult, op1=OP.subtract)
                nc.scalar.activation(out=stat_sb[:, :, :],
                                     in_=stat_sb[:, :, :],
                                     func=AF.Ln, bias=eps_sb[:, :], scale=1.0)
                nc.scalar.activation(out=alpha_T[:, :, isl],
                                     in_=stat_sb[:, :, :],
                                     func=AF.Exp, scale=-0.5)

                for h in range(H):
                    hp, hb = h // 2, 64 * (h % 2)
                    # alphaP[j | jt, i] = alpha_T * P_T[:, :, :, strip col h]
                    aP = apool.tile([128, JT, BLK], bf16, tag="aP")
                    nc.gpsimd.tensor_mul(
                        out=aP[:, :, :],
                        in0=alpha_T[:, :, i0:i0 + BLK],
                        in1=P_T[:, :, i0:i0 + BLK, h])
                    # logits = qk/8 + alphaP, two j-tiles per PSUM bank
                    for jp in range(JT // 2):
                        qk2 = ps.tile([128, 2, BLK], f32, tag="bank",
                                      name="qk2")
                        for q_ in range(2):
                            jt = 2 * jp + q_
                            nc.tensor.matmul(
                                out=qk2[:, q_, :],
                                lhsT=kT_sb[hb:hb + 64, hp,
                                           128 * jt:128 * (jt + 1)],
                                rhs=qT_sb[hb:hb + 64, hp, i0:i0 + BLK],
                                start=True, stop=True)
                        nc.vector.scalar_tensor_tensor(
                            out=aP[:, 2 * jp:2 * jp + 2, :],
                            in0=qk2[:, :, :], scalar=INV_SQRT_D,
                            in1=aP[:, 2 * jp:2 * jp + 2, :],
                            op0=OP.mult, op1=OP.add)
                    if mask_ones:
                        nc.scalar.activation(out=aP[:, :, :], in_=aP[:, :, :],
                                             func=AF.Exp)
                    else:
                        for jt in range(JT):
                            nc.scalar.activation(out=aP[:, jt, :],
                                                 in_=aP[:, jt, :],
                                                 func=AF.Exp,
                                                 bias=mb_T[:, jt:jt + 1],
                                                 scale=1.0)
                    # PV with ones-column: den lands at psum row 64
                    o_un = ps.tile([128, BLK], f32, tag="bank", name="o_un")
                    for jt in range(JT):
                        nc.tensor.matmul(out=o_un[0:E, :],
                                         lhsT=vaug[:, jt, E * h:E * (h + 1)],
                                         rhs=aP[:, jt, :],
                                         start=(jt == 0), stop=(jt == JT - 1))
                    nc.scalar.copy(out=den_sb[D:D + 1, h, :],
                                   in_=o_un[D:D + 1, :])
                    nc.vector.tensor_copy(out=o_sb[0:D, h, :],
                                          in_=o_un[0:D, :])

                # 1/den broadcast via DRAM round trip, fused with gate
                nc.sync.dma_start(
                    out=bass.AP(tensor=rden_dram, offset=blk * H * BLK,
                                ap=[[BLK, H], [1, BLK]]),
                    in_=den_sb[D:D + 1, :, :])
                nc.scalar.dma_start(
                    out=rden_t[0:64, :, :],
                    in_=bass.AP(tensor=rden_dram, offset=blk * H * BLK,
                                ap=[[0, 64], [2 * BLK, CT], [1, BLK]]))
                nc.scalar.dma_start(
                    out=rden_t[64:128, :, :],
                    in_=bass.AP(tensor=rden_dram, offset=blk * H * BLK + BLK,
                                ap=[[0, 64], [2 * BLK, CT], [1, BLK]]))
                # goT pre-gate: restore (h,d)-on-partitions layout via two
                # partition-shifting SBUF DMAs (even h -> 0:64, odd -> 64:128)
                goT = opool.tile([128, CT, BLK], bf16, tag="goT")
                nc.sync.dma_start(out=goT[0:64, :, :],
                                  in_=o_sb[:, 0:H:2, :])
                nc.scalar.dma_start(out=goT[64:128, :, :],
                                    in_=o_sb[:, 1:H:2, :])
                nc.vector.reciprocal(out=rden_t[:, :, :], in_=rden_t[:, :, :])
                nc.vector.tensor_mul(out=rden_t[:, :, :], in0=rden_t[:, :, :],
                                     in1=gT_sb[:, :, i0:i0 + BLK])
                nc.vector.tensor_mul(out=goT[:, :, :], in0=goT[:, :, :],
                                     in1=rden_t[:, :, :])
                # ---- output projection ----
                for f in range(CT):
                    o_ps = ps.tile([128, BLK], f32, tag="bank", name="o_ps")
                    for c in range(CT):
                        nc.tensor.matmul(
                            out=o_ps[:, :],
                            lhsT=Wo_sb[:, c, 128 * f:128 * (f + 1)],
                            rhs=goT[:, c, :],
                            start=(c == 0), stop=(c == CT - 1))
                    ot = opool.tile([128, BLK], f32, tag="ot")
                    nc.vector.tensor_copy(out=ot[:, :], in_=o_ps[:, :])
                    odst = bass.AP(tensor=outT_d, offset=128 * f * IPC + i0,
                                   ap=[[IPC, 128], [1, BLK]])
                    nc.sync.dma_start(out=odst, in_=ot[:, :])

            for blk in range(NBLK):
                for g in range(GPB * blk, GPB * (blk + 1)):
                    z_group(g)
                attention(blk)
    nc.compile()
    return nc


def _get_prog(mask_ones=True):
    if mask_ones not in _prog_cache:
        _prog_cache[mask_ones] = _build(mask_ones)
    return _prog_cache[mask_ones]


def _in_maps(s, z, mask, k_in, Wq, bq, Wk, Wv, Wg, ln_g, ln_b, Wz, Wo):
    del ln_b  # constant along j after softmax -> drops out exactly
    bf = BF16
    Wcast = {w: a.astype(bf) for w, a in
             (("Wq", Wq), ("Wk", Wk), ("Wv", Wv), ("Wg", Wg), ("Wo", Wo))}
    mask_ones = bool(np.all(mask == 1.0))
    maps = []
    nsl = NCORES // B
    for c in range(NCORES):
        b, t = divmod(c, nsl)
        i0 = t * IPC
        m = {
            "zT": z[b, i0:i0 + IPC].transpose(0, 2, 1).reshape(
                IPC * CZ, N).astype(bf),
            "sT": s[b, i0:i0 + IPC].T.astype(bf),
            "kinT": k_in[b].T.astype(bf),
            "Wz": np.ascontiguousarray(Wz, dtype=np.float32),
            "ln_g": np.ascontiguousarray(
                ln_g.reshape(CZ, 1), dtype=np.float32),
            "bqT": np.ascontiguousarray(
                bq.reshape(CT, 128).T, dtype=np.float32),
        }
        m.update(Wcast)
        if not mask_ones:
            m["maskrow"] = np.ascontiguousarray(
                mask[b].reshape(1, N), dtype=np.float32)
        maps.append(m)
    return maps


def kernel(**inputs):
    _install_trace_shim()
    from concourse.bass_utils import run_bass_kernel_spmd
    inputs = {k: np.asarray(v) for k, v in inputs.items()}
    mask_ones = bool(np.all(inputs["mask"] == 1.0))
    nc = _get_prog(mask_ones)
    maps = _in_maps(**inputs)
    trace = os.environ.get("KBENCH_TRACE", "") == "1"
    res = run_bass_kernel_spmd(nc, maps, core_ids=list(range(NCORES)),
                               trace=trace)
    out = np.empty((B, N, CS), dtype=np.float32)
    nsl = NCORES // B
    for c in range(NCORES):
        b, t = divmod(c, nsl)
        out[b, t * IPC:(t + 1) * IPC, :] = res.results[c]["outT"].T
    if trace:
        print("HW exec time:", res.exec_time_ns, "ns")
    return out


# revision 23
# speedup vs baseline: 1.7098x; 1.2324x over previous
"""AttentionPairBias kernel for Trainium2, 8-core SPMD.

  q = (s @ Wq + bq); k = k_in @ Wk; v = k_in @ Wv          (B,N,H,D)
  bias = transpose(LayerNorm(z) @ Wz) -> (B,H,N,N)
  g = sigmoid(s @ Wg)
  p = softmax_j(q.k/sqrt(D) + bias + maskterm)
  out = (g * (p @ v)) @ Wo

Sharding: 8 cores = (batch b in {0,1}) x (4 slices of 192 query rows i).
Each core computes out[b, i0:i0+192, :] completely.  Inputs are uploaded
pre-cast to bf16 (halves HBM traffic; adds ~0.4% rel err vs 2e-2 budget).

Device structure per core:
  - LayerNorm folded into the Wz projection (W'' = ln_g*Wz - colmean/128;
    ln_b and per-row constants drop out of softmax_j).  Per 4 z-rows, one
    pair of PSUM tiles holds 32-partition strips, strip rl carrying:
      rows 0..15 = W''^T z (pre-bias P), row 16 = sum_c z, row 17 = sum_c z^2
    (P/sumz via lhsT=Waug[:,0:17] over z; sum z^2 via lhsT=ones over z^2).
  - PSUM strip -> SBUF bf16 -> XBAR dma-transpose into P_T[j | g, strip],
    so softmax runs with j on partitions:
      * p[j,i] is directly the PV lhsT operand: no PE transposes of p
      * logits = qk/8 + alpha*P via strided DVE/Pool ops
      * softmax denominator rides a ones-column inside vaug (even heads
        [v|1] -> den at psum row 64; odd heads [1|v] based at row 63)
  - normalization by the denominator happens post-PV (196K elems instead
    of 2.4M), fused with the sigmoid gate via a DRAM-broadcast of 1/den.
  - attention runs in two 96-row blocks so block 0 overlaps the z stream
    of block 1; the Wo projection of block 0 overlaps block 1 attention.
"""

import os
import sys
import types

import numpy as np
import ml_dtypes

B, N, CS, CZ, H, D = 2, 768, 1024, 128, 16, 64
NCORES = 8
IPC = N // 4            # 192 query rows per core
NBLK = 2
BLK = IPC // NBLK       # 96 rows per attention block
GRP = 4                 # z rows per PSUM-strip group
GPB = BLK // GRP        # 24 groups per block
NG = IPC // GRP         # 48 groups
JT = N // 128           # 6 j tiles
CT = CS // 128          # 8 c_s tiles
E = D + 1               # 65: head slot width in vaug (v + ones column)
LN_EPS = 1e-5
INV_SQRT_D = 0.125
INF = 1000000.0

_prog_cache = {}
BF16 = ml_dtypes.bfloat16


def _install_trace_shim():
    """Provide antenv.axon_hooks (NTFF profiling) if the image lacks it."""
    try:
        import antenv.axon_hooks  # noqa: F401
        return
    except ImportError:
        pass
    try:
        import trn_agent_boot.trn_boot as tb
        hook = tb._ntff_profile_via_ctypes('/opt/axon/libaxon_pjrt.so')
        mod = types.ModuleType("antenv.axon_hooks")
        mod.get_axon_ntff_profile_hook = lambda: hook
        sys.modules["antenv.axon_hooks"] = mod
    except Exception:
        pass


def _build(mask_ones=True):
    import contextlib
    import concourse.bass as bass
    import concourse.tile as tile
    from concourse import bacc, mybir

    f32 = mybir.dt.float32
    bf16 = mybir.dt.bfloat16
    AF = mybir.ActivationFunctionType
    OP = mybir.AluOpType

    nc = bacc.Bacc("TRN2", target_bir_lowering=False, debug=False,
                   enable_asserts=False, num_devices=NCORES)

    # ---- DRAM I/O (bf16 inputs pre-cast on host) ----
    zT_d = nc.dram_tensor("zT", [IPC * CZ, N], bf16, kind="ExternalInput")
    sT_d = nc.dram_tensor("sT", [CS, IPC], bf16, kind="ExternalInput")
    kinT_d = nc.dram_tensor("kinT", [CS, N], bf16, kind="ExternalInput")
    W_d = {w: nc.dram_tensor(w, [CS, CS], bf16, kind="ExternalInput")
           for w in ("Wq", "Wk", "Wv", "Wg", "Wo")}
    Wz_d = nc.dram_tensor("Wz", [CZ, H], f32, kind="ExternalInput")
    lng_d = nc.dram_tensor("ln_g", [CZ, 1], f32, kind="ExternalInput")
    bqT_d = nc.dram_tensor("bqT", [128, CT], f32, kind="ExternalInput")
    if not mask_ones:
        mask_d = nc.dram_tensor("maskrow", [1, N], f32, kind="ExternalInput")
    outT_d = nc.dram_tensor("outT", [CS, IPC], f32, kind="ExternalOutput")

    with tile.TileContext(nc) as tc:
        # ---------------- persistent SBUF ----------------
        Waug = nc.alloc_sbuf_tensor("Waug", [128, 32], bf16)
        # Saug: zeros except col 17 = ones; the z^2 matmul writes strip rows
        # 0..17 (rows 0..16 zero, row 17 = sum z^2) BEFORE the P matmul
        # overwrites rows 0..16 with P / sum z.
        Saug = nc.alloc_sbuf_tensor("Saug", [128, 18], bf16)
        Wz_sb = nc.alloc_sbuf_tensor("Wz_sb", [128, H], f32)
        lng_sb = nc.alloc_sbuf_tensor("lng_sb", [128, 1], f32)
        Wp_sb = nc.alloc_sbuf_tensor("Wp_sb", [128, H], f32)
        G_sb = nc.alloc_sbuf_tensor("G_sb", [128, H], f32)
        Grow_sb = nc.alloc_sbuf_tensor("Grow_sb", [1, H], f32)
        bqT_sb = nc.alloc_sbuf_tensor("bqT_sb", [128, CT], f32)
        eps_sb = nc.alloc_sbuf_tensor("eps_sb", [128, 1], f32)

        sT_sb = nc.alloc_sbuf_tensor("sT_sb", [128, CT, IPC], bf16)
        kinT_sb = nc.alloc_sbuf_tensor("kinT_sb", [128, CT, N], bf16)
        qT_sb = nc.alloc_sbuf_tensor("qT_sb", [128, CT, IPC], bf16)
        gT_sb = nc.alloc_sbuf_tensor("gT_sb", [128, CT, IPC], bf16)
        kT_sb = nc.alloc_sbuf_tensor("kT_sb", [128, CT, N], bf16)
        vaug = nc.alloc_sbuf_tensor("vaug", [128, JT, H * E], bf16)
        # P_T[j | jt, i, strip]: strip col h in 0..15 = head-h pre-bias P,
        # col 16 = sum_c z, col 17 = sum_c z^2 (stats consumed in j-layout)
        P_T = nc.alloc_sbuf_tensor("P_T", [128, JT, IPC, 32], bf16)
        alpha_T = nc.alloc_sbuf_tensor("alpha_T", [128, JT, IPC], bf16)
        stat_sb = nc.alloc_sbuf_tensor("stat_sb", [128, JT, BLK], f32)
        id_sb = nc.alloc_sbuf_tensor("id_sb", [128, 128], bf16)
        den_sb = nc.alloc_sbuf_tensor("den_sb", [128, H, BLK], f32)
        o_sb = nc.alloc_sbuf_tensor("o_sb", [64, H, BLK], bf16)
        rden_t = nc.alloc_sbuf_tensor("rden_t", [128, CT, BLK], f32)
        rden_dram = nc.dram_tensor("rden_dram", [NBLK, H * BLK], f32,
                                   kind="Internal")
        Grow_dram = nc.dram_tensor("Grow_dram", [H], f32, kind="Internal")
        if not mask_ones:
            mrow_sb = nc.alloc_sbuf_tensor("mrow_sb", [1, N], f32)
            mb_row = nc.alloc_sbuf_tensor("mb_row", [1, N], f32)
            mb_dram = nc.dram_tensor("mb_dram", [N], f32, kind="Internal")
            mb_T = nc.alloc_sbuf_tensor("mb_T", [128, JT], f32)

        ctx = contextlib.ExitStack()
        with ctx:
            ps = ctx.enter_context(tc.tile_pool(name="ps", bufs=5,
                                                space="PSUM"))
            aps = ctx.enter_context(tc.tile_pool(name="aps", bufs=3,
                                                 space="PSUM"))
            zpool = ctx.enter_context(tc.tile_pool(name="zs", bufs=3))
            sqpool = ctx.enter_context(tc.tile_pool(name="sq", bufs=2))
            cpool = ctx.enter_context(tc.tile_pool(name="cs", bufs=4))
            apool = ctx.enter_context(tc.tile_pool(name="apl", bufs=3))
            wpool = ctx.enter_context(tc.tile_pool(name="wp", bufs=1))
            opool = ctx.enter_context(tc.tile_pool(name="op", bufs=3))

            # ---------------- constants / Waug prep ----------------
            nc.sync.dma_start(out=bqT_sb[:, :], in_=bqT_d.ap())
            nc.sync.dma_start(out=Wz_sb[:, :], in_=Wz_d.ap())
            nc.sync.dma_start(out=lng_sb[:, :], in_=lng_d.ap())
            nc.vector.memset(eps_sb[:, :], LN_EPS)
            nc.vector.memset(Saug[:, :], 0.0)
            nc.vector.memset(Saug[:, 17:18], 1.0)
            nc.vector.memset(Waug[:, :], 0.0)
            nc.vector.memset(vaug[:, :, :], 1.0)
            from concourse.masks import make_identity
            make_identity(nc, id_sb[:, :])
            # W' = ln_g*Wz ; G = colsum(W') ; W'' = W' - G/128
            nc.vector.tensor_scalar_mul(out=Wp_sb[:, :], in0=Wz_sb[:, :],
                                        scalar1=lng_sb[:, :])
            nc.vector.tensor_copy(out=Waug[:, 0:H], in_=Wp_sb[:, :])
            nc.vector.memset(Waug[:, H:H + 1], 1.0)
            G_ps = aps.tile([1, H], f32, tag="abank")
            nc.tensor.matmul(out=G_ps[:, :], lhsT=Waug[:, H:H + 1],
                             rhs=Waug[:, 0:H], start=True, stop=True)
            nc.vector.tensor_copy(out=Grow_sb[:, :], in_=G_ps[:, :])
            nc.sync.dma_start(out=Grow_dram.ap(), in_=Grow_sb[:, :])
            nc.sync.dma_start(
                out=G_sb[:, :],
                in_=bass.AP(tensor=Grow_dram, offset=0, ap=[[0, 128], [1, H]]))
            nc.vector.scalar_tensor_tensor(out=Wp_sb[:, :], in0=G_sb[:, :],
                                           scalar=-1.0 / 128.0,
                                           in1=Wp_sb[:, :],
                                           op0=OP.mult, op1=OP.add)
            nc.vector.tensor_copy(out=Waug[:, 0:H], in_=Wp_sb[:, :])
            # Waug col 16 stays all-ones: sum_c z rides the P matmul.
            if not mask_ones:
                nc.sync.dma_start(out=mrow_sb[:, :], in_=mask_d.ap())
                nc.vector.tensor_scalar(out=mb_row[:, :], in0=mrow_sb[:, :],
                                        scalar1=1.0, scalar2=INF,
                                        op0=OP.subtract, op1=OP.mult)
                nc.sync.dma_start(out=mb_dram.ap(), in_=mb_row[:, :])
                nc.sync.dma_start(
                    out=mb_T[:, :],
                    in_=bass.AP(tensor=mb_dram, offset=0,
                                ap=[[1, 128], [128, JT]]))

            # ---------------- activations / projections ----------------
            nc.scalar.dma_start(
                out=sT_sb[:, :, :],
                in_=sT_d.ap().rearrange("(t p) i -> p t i", p=128))
            nc.scalar.dma_start(
                out=kinT_sb[:, :, :],
                in_=kinT_d.ap().rearrange("(t p) j -> p t j", p=128))

            def load_w(w):
                t = wpool.tile([128, CT, CS], bf16, tag="w")
                nc.sync.dma_start(
                    out=t[:, :, :],
                    in_=W_d[w].ap().rearrange("(t p) f -> p t f", p=128))
                return t

            Wg_sb = load_w("Wg")
            for f in range(CT):
                g_ps = aps.tile([128, IPC], f32, tag="abank")
                for c in range(CT):
                    nc.tensor.matmul(out=g_ps[:, :],
                                     lhsT=Wg_sb[:, c, 128 * f:128 * (f + 1)],
                                     rhs=sT_sb[:, c, :],
                                     start=(c == 0), stop=(c == CT - 1))
                nc.scalar.activation(out=gT_sb[:, f, :], in_=g_ps[:, :],
                                     func=AF.Sigmoid)
            Wq_sb = load_w("Wq")
            for f in range(CT):
                q_ps = aps.tile([128, IPC], f32, tag="abank")
                for c in range(CT):
                    nc.tensor.matmul(out=q_ps[:, :],
                                     lhsT=Wq_sb[:, c, 128 * f:128 * (f + 1)],
                                     rhs=sT_sb[:, c, :],
                                     start=(c == 0), stop=(c == CT - 1))
                nc.vector.tensor_scalar_add(out=qT_sb[:, f, :], in0=q_ps[:, :],
                                            scalar1=bqT_sb[:, f:f + 1])
            Wk_sb = load_w("Wk")
            for f in range(CT):
                for hf in range(2):
                    k_ps = aps.tile([128, 384], f32, tag="abank")
                    for c in range(CT):
                        nc.tensor.matmul(
                            out=k_ps[:, :],
                            lhsT=Wk_sb[:, c, 128 * f:128 * (f + 1)],
                            rhs=kinT_sb[:, c, 384 * hf:384 * (hf + 1)],
                            start=(c == 0), stop=(c == CT - 1))
                    if (f + hf) % 2 == 0:
                        nc.vector.tensor_copy(
                            out=kT_sb[:, f, 384 * hf:384 * (hf + 1)],
                            in_=k_ps[:, :])
                    else:
                        nc.scalar.copy(
                            out=kT_sb[:, f, 384 * hf:384 * (hf + 1)],
                            in_=k_ps[:, :])
            Wv_sb = load_w("Wv")
            for jt in range(JT):
                for vh in range(2):
                    v_ps = aps.tile([128, 512], f32, tag="abank")
                    for c in range(CT):
                        nc.tensor.matmul(
                            out=v_ps[:, :],
                            lhsT=kinT_sb[:, c, 128 * jt:128 * (jt + 1)],
                            rhs=Wv_sb[:, c, 512 * vh:512 * (vh + 1)],
                            start=(c == 0), stop=(c == CT - 1))
                    # scatter 8 heads into vaug slots [v | ones]
                    base = 8 * vh * E
                    dst = vaug[:, jt, base:base + 8 * E].rearrange(
                        "p (e c) -> p e c", c=E)
                    src = v_ps[:, :].rearrange("p (e c) -> p e c", c=D)
                    nc.vector.tensor_copy(out=dst[:, :, 0:D], in_=src[:, :, :])

            Wo_sb = load_w("Wo")   # prefetched during z stream

            # ---------------- z stream ----------------
            def z_group(g):
                blk = g // GPB
                zTt = zpool.tile([128, GRP, N], bf16, tag="zTt")
                zsrc = bass.AP(tensor=zT_d, offset=(GRP * g) * CZ * N,
                               ap=[[N, 128], [CZ * N, GRP], [1, N]])
                (nc.sync if g % 2 == 0 else nc.scalar).dma_start(
                    out=zTt[:, :, :], in_=zsrc)
                zsq = sqpool.tile([128, GRP, N], bf16, tag="zsq")
                if g % 3 != 2:
                    nc.vector.tensor_mul(out=zsq[:, :, :], in0=zTt[:, :, :],
                                         in1=zTt[:, :, :])
                else:
                    nc.scalar.activation(out=zsq[:, :, :], in_=zTt[:, :, :],
                                         func=AF.Square)
                Ppk = [ps.tile([128, 384], f32, tag="bank", name=f"Ppk{hf}")
                       for hf in range(2)]
                # z^2 matmuls first (rows 0..17, row 17 = sum z^2), then P
                # overwrites rows 0..16 (row 16 = sum z); grouping by lhsT
                # keeps the stationary weights loaded.
                for hf in range(2):
                    for rl in range(GRP):
                        nc.tensor.matmul(
                            out=Ppk[hf][32 * rl:32 * rl + H + 2, :],
                            lhsT=Saug[:, :],
                            rhs=zsq[:, rl, 384 * hf:384 * (hf + 1)],
                            start=True, stop=True,
                            tile_position=(0, 32 * rl))
                for hf in range(2):
                    for rl in range(GRP):
                        nc.tensor.matmul(
                            out=Ppk[hf][32 * rl:32 * rl + H + 1, :],
                            lhsT=Waug[:, 0:H + 1],
                            rhs=zTt[:, rl, 384 * hf:384 * (hf + 1)],
                            start=True, stop=True,
                            tile_position=(0, 32 * rl))
                PT_ps = ps.tile([128, JT, 128], bf16, tag="bank", name="PT_ps")
                for hf in range(2):
                    PSsb = cpool.tile([128, 384], bf16, tag="PSsb")
                    nc.scalar.copy(out=PSsb[:, :], in_=Ppk[hf][:, :])
                    # PE transpose -> PT_ps[j | jt, strip]
                    for jtl in range(3):
                        nc.tensor.transpose(
                            out=PT_ps[:, 3 * hf + jtl, :],
                            in_=PSsb[:, 128 * jtl:128 * (jtl + 1)],
                            identity=id_sb[:, :])
                nc.vector.tensor_copy(
                    out=P_T[:, :, GRP * g:GRP * (g + 1), :],
                    in_=PT_ps[:, :, :])

            # ---------------- attention block ----------------
            def attention(blk):
                i0 = BLK * blk
                isl = slice(i0, i0 + BLK)
                # alpha = rstd per (i, j) computed in j-layout from P_T
                # cols 16 (sum z) and 17 (sum z^2)
                nc.vector.scalar_tensor_tensor(
                    out=stat_sb[:, :, :], in0=P_T[:, :, isl, 16],
                    scalar=1.0 / (128.0 * 128.0), in1=P_T[:, :, isl, 16],
                    op0=OP.mult, op1=OP.mult)
                nc.vector.scalar_tensor_tensor(
                    out=stat_sb[:, :, :], in0=P_T[:, :, isl, 17],
                    scalar=1.0 / 128.0, in1=stat_sb[:, :, :],
                    op0=OP.mult, op1=OP.subtract)
                nc.scalar.activation(out=stat_sb[:, :, :],
                                     in_=stat_sb[:, :, :],
                                     func=AF.Ln, bias=eps_sb[:, :], scale=1.0)
                nc.scalar.activation(out=alpha_T[:, :, isl],
                                     in_=stat_sb[:, :, :],
                                     func=AF.Exp, scale=-0.5)

                for h in range(H):
                    hp, hb = h // 2, 64 * (h % 2)
                    # alphaP[j | jt, i] = alpha_T * P_T[:, :, :, strip col h]
                    aP = apool.tile([128, JT, BLK], bf16, tag="aP")
                    nc.vector.tensor_mul(
                        out=aP[:, :, :],
                        in0=alpha_T[:, :, i0:i0 + BLK],
                        in1=P_T[:, :, i0:i0 + BLK, h])
                    # logits = qk/8 + alphaP, two j-tiles per PSUM bank
                    for jp in range(JT // 2):
                        qk2 = aps.tile([128, 2, BLK], f32, tag="abank",
                                      name="qk2")
                        for q_ in range(2):
                            jt = 2 * jp + q_
                            nc.tensor.matmul(
                                out=qk2[:, q_, :],
                                lhsT=kT_sb[hb:hb + 64, hp,
                                           128 * jt:128 * (jt + 1)],
                                rhs=qT_sb[hb:hb + 64, hp, i0:i0 + BLK],
                                start=True, stop=True)
                        nc.vector.scalar_tensor_tensor(
                            out=aP[:, 2 * jp:2 * jp + 2, :],
                            in0=qk2[:, :, :], scalar=INV_SQRT_D,
                            in1=aP[:, 2 * jp:2 * jp + 2, :],
                            op0=OP.mult, op1=OP.add)
                    if mask_ones:
                        nc.scalar.activation(out=aP[:, :, :], in_=aP[:, :, :],
                                             func=AF.Exp)
                    else:
                        for jt in range(JT):
                            nc.scalar.activation(out=aP[:, jt, :],
                                                 in_=aP[:, jt, :],
                                                 func=AF.Exp,
                                                 bias=mb_T[:, jt:jt + 1],
                                                 scale=1.0)
                    # PV with ones-column: den lands at psum row 64
                    o_un = aps.tile([128, BLK], f32, tag="abank", name="o_un")
                    for jt in range(JT):
                        nc.tensor.matmul(out=o_un[0:E, :],
                                         lhsT=vaug[:, jt, E * h:E * (h + 1)],
                                         rhs=aP[:, jt, :],
                                         start=(jt == 0), stop=(jt == JT - 1))
                    nc.scalar.copy(out=den_sb[D:D + 1, h, :],
                                   in_=o_un[D:D + 1, :])
                    nc.vector.tensor_copy(out=o_sb[0:D, h, :],
                                          in_=o_un[0:D, :])

                # 1/den broadcast via DRAM round trip, fused with gate
                nc.gpsimd.dma_start(
                    out=bass.AP(tensor=rden_dram, offset=blk * H * BLK,
                                ap=[[BLK, H], [1, BLK]]),
                    in_=den_sb[D:D + 1, :, :])
                nc.gpsimd.dma_start(
                    out=rden_t[0:64, :, :],
                    in_=bass.AP(tensor=rden_dram, offset=blk * H * BLK,
                                ap=[[0, 64], [2 * BLK, CT], [1, BLK]]))
                nc.gpsimd.dma_start(
                    out=rden_t[64:128, :, :],
                    in_=bass.AP(tensor=rden_dram, offset=blk * H * BLK + BLK,
                                ap=[[0, 64], [2 * BLK, CT], [1, BLK]]))
                # goT pre-gate: restore (h,d)-on-partitions layout via two
                # partition-shifting SBUF DMAs (even h -> 0:64, odd -> 64:128)
                goT = opool.tile([128, CT, BLK], bf16, tag="goT")
                nc.gpsimd.dma_start(out=goT[0:64, :, :],
                                    in_=o_sb[:, 0:H:2, :])
                nc.gpsimd.dma_start(out=goT[64:128, :, :],
                                    in_=o_sb[:, 1:H:2, :])
                nc.vector.reciprocal(out=rden_t[:, :, :], in_=rden_t[:, :, :])
                nc.vector.tensor_mul(out=rden_t[:, :, :], in0=rden_t[:, :, :],
                                     in1=gT_sb[:, :, i0:i0 + BLK])
                nc.vector.tensor_mul(out=goT[:, :, :], in0=goT[:, :, :],
                                     in1=rden_t[:, :, :])
                # ---- output projection ----
                for f in range(CT):
                    o_ps = aps.tile([128, BLK], f32, tag="abank", name="o_ps")
                    for c in range(CT):
                        nc.tensor.matmul(
                            out=o_ps[:, :],
                            lhsT=Wo_sb[:, c, 128 * f:128 * (f + 1)],
                            rhs=goT[:, c, :],
                            start=(c == 0), stop=(c == CT - 1))
                    ot = opool.tile([128, BLK], f32, tag="ot")
                    nc.vector.tensor_copy(out=ot[:, :], in_=o_ps[:, :])
                    odst = bass.AP(tensor=outT_d, offset=128 * f * IPC + i0,
                                   ap=[[IPC, 128], [1, BLK]])
                    nc.gpsimd.dma_start(out=odst, in_=ot[:, :])

            for blk in range(NBLK):
                for g in range(GPB * blk, GPB * (blk + 1)):
                    z_group(g)
                attention(blk)
    nc.compile()
    return nc


def _get_prog(mask_ones=True):
    if mask_ones not in _prog_cache:
        _prog_cache[mask_ones] = _build(mask_ones)
    return _prog_cache[mask_ones]


def _in_maps(s, z, mask, k_in, Wq, bq, Wk, Wv, Wg, ln_g, ln_b, Wz, Wo):
    del ln_b  # constant along j after softmax -> drops out exactly
    bf = BF16
    Wcast = {w: a.astype(bf) for w, a in
             (("Wq", Wq), ("Wk", Wk), ("Wv", Wv), ("Wg", Wg), ("Wo", Wo))}
    mask_ones = bool(np.all(mask == 1.0))
    maps = []
    nsl = NCORES // B
    for c in range(NCORES):
        b, t = divmod(c, nsl)
        i0 = t * IPC
        m = {
            "zT": z[b, i0:i0 + IPC].transpose(0, 2, 1).reshape(
                IPC * CZ, N).astype(bf),
            "sT": s[b, i0:i0 + IPC].T.astype(bf),
            "kinT": k_in[b].T.astype(bf),
            "Wz": np.ascontiguousarray(Wz, dtype=np.float32),
            "ln_g": np.ascontiguousarray(
                ln_g.reshape(CZ, 1), dtype=np.float32),
            "bqT": np.ascontiguousarray(
                bq.reshape(CT, 128).T, dtype=np.float32),
        }
        m.update(Wcast)
        if not mask_ones:
            m["maskrow"] = np.ascontiguousarray(
                mask[b].reshape(1, N), dtype=np.float32)
        maps.append(m)
    return maps


def kernel(**inputs):
    _install_trace_shim()
    from concourse.bass_utils import run_bass_kernel_spmd
    inputs = {k: np.asarray(v) for k, v in inputs.items()}
    mask_ones = bool(np.all(inputs["mask"] == 1.0))
    nc = _get_prog(mask_ones)
    maps = _in_maps(**inputs)
    trace = os.environ.get("KBENCH_TRACE", "") == "1"
    res = run_bass_kernel_spmd(nc, maps, core_ids=list(range(NCORES)),
                               trace=trace)
    out = np.empty((B, N, CS), dtype=np.float32)
    nsl = NCORES // B
    for c in range(NCORES):
        b, t = divmod(c, nsl)
        out[b, t * IPC:(t + 1) * IPC, :] = res.results[c]["outT"].T
    if trace:
        print("HW exec time:", res.exec_time_ns, "ns")
    return out
